# revision 1
# baseline (speedup 1.0000x reference)
"""Trainium2 Bass kernel for nn_BatchHighOrderActivation.

Math: out[b,i,o] = sum_k coef_k * params[i, idx_k, o]  (sorted-diff coefs,
reverse-cumsum subset masks).  Rewritten gather-free as

    out[b,i,:] = sum_{m=1..15} w_m[b,i] * params[i, m, :]
    w_m = relu( min_{j in m} X_j  -  max_{j not in m} X_j )   (m != 15)
    w_15 = min4 = relu(min4) - relu(-min4)  (split across two relu slots)

Per core (batch-sharded 8 ways, 1024 batch rows each), per 128-row b-tile:
  1. SP HWDGE loads X tile (host pre-casts X to bf16).
  2. Pool: deinterleave -> 4 planes X_j (strided tensor_copy).
  3. DVE: the full min/max lattice (pmin/pmax/tmin/tmax/s14) -- GPSIMD has
     no tensor-tensor min/max ucode, so these must stay on DVE (2x bf16).
  4. Pool: the 14 slot subtractions (GPSIMD supports subtract; all operand
     access patterns ascending -- GPSIMD also rejects negative-stride and
     broadcast APs) + the s15 negation.
  5. PE:  transpose W group-tiles ([128b x 128q]) -> PSUM bf16.
  6. ACT/DVE: relu-evacuate PSUM -> lhsT tiles [128q, 128b] bf16.
     (GPSIMD cannot touch PSUM.)
  7. PE:  matmul lhsT.T @ PD[g] (PD = block-diag P, K=q) -> PSUM fp32.
  8. ACT: cast-copy PSUM fp32 -> SBUF bf16.
  9. SP:  DMA out bf16 (host upcasts to fp32 after gather).

Engine busy budget per core (v1 cost model): DVE ~136us (lattice + relu
share), ACT ~136us (relu share + out-evacs), Pool ~130us (deint + subs),
SP ~82us DMA, PE ~82us.
"""

import sys

for _p in ("/opt/trn_rl_repo", "/root/.axon_site/_ro/trn_rl_repo"):
    if _p not in sys.path:
        sys.path.append(_p)

import numpy as np
import ml_dtypes

B, I, A, O = 8192, 1024, 4, 8
NCORES = 8
BC = B // NCORES          # batch rows per core
NG = I // 8               # 128 groups of 8 i-rows
NSLOT = 16

# slot order chosen so merged double-width subs write adjacent slots:
# s0..3 singles {0}{1}{2}{3}; s4..9 pair-masks in PAIRS order; s10..13
# triples ordered by excluded coordinate; s14/15 = +/- full-set (mask 15)
SLOT_MASKS = [1, 2, 4, 8, 3, 12, 5, 10, 9, 6, 14, 13, 11, 7]

_CACHE = {}


def _build_pd(params: np.ndarray) -> np.ndarray:
    """Block-diagonal P table: PD[q = s*8 + i_sub, g, n = i_sub*8 + o]."""
    Pt = np.empty((I, NSLOT, O), np.float32)
    for s, m in enumerate(SLOT_MASKS):
        Pt[:, s, :] = params[:, m, :]
    Pt[:, 14, :] = params[:, 15, :]
    Pt[:, 15, :] = -params[:, 15, :]

    PD = np.zeros((128, NG, 64), np.float32)
    for s in range(NSLOT):
        for isub in range(8):
            PD[s * 8 + isub, :, isub * 8:(isub + 1) * 8] = Pt[
                np.arange(NG) * 8 + isub, s, :
            ]
    return PD.reshape(128, NG * 64).astype(ml_dtypes.bfloat16)


def _build_bass():
    import concourse.bass as bass
    import concourse.mybir as mybir
    import concourse.tile as tile
    from concourse import bacc
    from concourse.masks import make_identity

    f32 = mybir.dt.float32
    wdt = mybir.dt.bfloat16

    nc = bacc.Bacc(None)
    Xp = nc.declare_dram_parameter("X", [BC, I, A], wdt, isOutput=False)
    PDp = nc.declare_dram_parameter("PD", [128, NG * 64], wdt, isOutput=False)
    OUTp = nc.declare_dram_parameter("OUT", [BC, I, O], wdt, isOutput=True)

    AF = mybir.ActivationFunctionType
    ALU = mybir.AluOpType

    IH = I // 2    # i-half extent per lattice pass

    with tile.TileContext(nc) as tc:
        with (
            tc.tile_pool(name="consts", bufs=1) as consts,
            tc.tile_pool(name="xin", bufs=5) as xin_pool,
            tc.tile_pool(name="xj", bufs=3) as xj_pool,
            tc.tile_pool(name="scr", bufs=2) as scr_pool,
            tc.tile_pool(name="w", bufs=3) as w_pool,
            tc.tile_pool(name="lh", bufs=4) as lh_pool,
            tc.tile_pool(name="ot", bufs=5) as ot_pool,
            tc.tile_pool(name="psT", bufs=2, space="PSUM") as psT_pool,
            tc.tile_pool(name="psO", bufs=2, space="PSUM") as psO_pool,
        ):
            ident = consts.tile([128, 128], wdt)
            make_identity(nc, ident)
            pd_sb = consts.tile([128, NG * 64], wdt)

            NT = BC // 128
            # X tiles prefetched with lookahead >= 2 so the loads run ahead
            # of the same-tile OUT DMAs in the SP queue's program order
            xts = {}

            def load_x(tt):
                if tt >= NT or tt in xts:
                    return
                bs = slice(tt * 128, (tt + 1) * 128)
                xt = xin_pool.tile([128, I, A], wdt)
                # tile 0 loads in quarters so the first 256-row chunk's
                # deinterleave starts as early as possible
                step = I // 4 if tt == 0 else I // 2
                for i0 in range(0, I, step):
                    nc.sync.dma_start(
                        out=xt[:, i0:i0 + step, :], in_=Xp[bs, i0:i0 + step, :]
                    )
                xts[tt] = xt

            load_x(0)
            load_x(1)
            # PD load on the ACT queue: fills ACT's pipeline-warmup idle and
            # keeps the SP queue free for the first two X tiles
            nc.scalar.dma_start(out=pd_sb[:], in_=PDp[:])

            # flat chunk list across tiles; first tile ramps up in 256-row
            # chunks (shorter pipeline fill); last tile tapers off likewise
            # so the final post-lattice PE/evac chain (the tail) is shorter
            all_chunks = []
            for t in range(NT):
                if t == 0:
                    tch = [(0, 256), (256, 256), (512, IH)]
                elif t == NT - 1:
                    tch = [(0, IH), (IH, 256), (IH + 256, 256)]
                else:
                    tch = [(0, IH), (IH, IH)]
                for ic0, ilen in tch:
                    all_chunks.append((t, ic0, ilen))

            # deinterleave on Pool: strided read (i,j)->(j,i); hoisted one
            # chunk ahead so DVE's next-chunk mins never wait on Pool's
            # subtraction backlog
            xjs = {}

            def deint(ci):
                if ci >= len(all_chunks) or ci in xjs:
                    return
                ct, cic0, cilen = all_chunks[ci]
                xj = xj_pool.tile([128, A, cilen], wdt)
                nc.gpsimd.tensor_copy(
                    out=xj[:],
                    in_=xts[ct][:, cic0:cic0 + cilen, :].rearrange(
                        "p i j -> p j i"
                    ),
                )
                xjs[ci] = xj
                if ci + 1 >= len(all_chunks) or all_chunks[ci + 1][0] != ct:
                    xts.pop(ct)  # last chunk of this tile: release xt

            # DVE min-side (pmin trio + merged tmin pair), software-pipelined
            # one chunk ahead of the max-side + Pool subs.
            mins = {}

            def emit_mins(ci):
                if ci >= len(all_chunks) or ci in mins:
                    return
                _, _, cilen = all_chunks[ci]
                cxj = xjs[ci]
                pmin = scr_pool.tile([128, 6, cilen], wdt, tag="pmin")
                tmin = scr_pool.tile([128, 4, cilen], wdt, tag="tmin")
                #  pmin[0:2]=[min01,min23] [2:4]=[min02,min13] [4:6]=[min03,min12]
                nc.vector.tensor_tensor(
                    pmin[:, 0:2], cxj[:, 0::2], cxj[:, 1::2], ALU.min
                )
                nc.vector.tensor_tensor(
                    pmin[:, 2:4], cxj[:, 0:2], cxj[:, 2:4], ALU.min
                )
                nc.vector.tensor_tensor(
                    pmin[:, 4:6], cxj[:, 0:2], cxj[:, 3:1:-1], ALU.min
                )
                # tmin_e = min over X\{e}: tmin[0:2] = min(min23, [x1, x0]);
                # tmin[2:4] = min(min01, [x3, x2])
                nc.vector.tensor_tensor(
                    tmin[:, 0:2],
                    pmin[:, 1:2].broadcast_to([128, 2, cilen]),
                    cxj[:, 1::-1], ALU.min,
                )
                nc.vector.tensor_tensor(
                    tmin[:, 2:4],
                    pmin[:, 0:1].broadcast_to([128, 2, cilen]),
                    cxj[:, 3:1:-1], ALU.min,
                )
                mins[ci] = (pmin, tmin)

            deint(0)
            emit_mins(0)
            for ci, (t, ic0, ilen) in enumerate(all_chunks):
                bsl = slice(t * 128, (t + 1) * 128)
                if ci + 1 < len(all_chunks) and all_chunks[ci + 1][0] != t:
                    load_x(t + 2)
                    load_x(t + 3)
                xj = xjs.pop(ci)
                pmin, tmin = mins.pop(ci)

                pmax = scr_pool.tile([128, 6, ilen], wdt, tag="pmax")
                tmax = scr_pool.tile([128, 4, ilen], wdt, tag="tmax")
                # W grouped: free = (group g, q = s*8 + i_sub)
                w = w_pool.tile([128, ilen // 8, NSLOT * 8], wdt)

                def wslot(s):
                    return w[:, :, s * 8:(s + 1) * 8]

                def grp(ap):
                    return ap.rearrange("p (g e) -> p g e", e=8)

                # DVE max side; pmax[k] = max over complement of the k-th
                # pair so pair-subs align ascending with pmin:
                #  pmax[0:2]=[max23,max01] [2:4]=[max13,max02] [4:6]=[max12,max03]
                nc.vector.tensor_tensor(
                    pmax[:, 0:2], xj[:, 2::-2], xj[:, 3::-2], ALU.max
                )
                nc.vector.tensor_tensor(
                    pmax[:, 2:4], xj[:, 1::-1], xj[:, 3:1:-1], ALU.max
                )
                nc.vector.tensor_tensor(
                    pmax[:, 4:6], xj[:, 1::-1], xj[:, 2:4], ALU.max
                )
                # tmax_e = max over X\{e}: tmax[0:2] = max(max23, [x1, x0]);
                # tmax[2:4] = max(max01, [x3, x2]).  pmax[0]=max23,
                # pmax[1]=max01.
                nc.vector.tensor_tensor(
                    tmax[:, 0:2],
                    pmax[:, 0:1].broadcast_to([128, 2, ilen]),
                    xj[:, 1::-1], ALU.max,
                )
                nc.vector.tensor_tensor(
                    tmax[:, 2:4],
                    pmax[:, 1:2].broadcast_to([128, 2, ilen]),
                    xj[:, 3:1:-1], ALU.max,
                )
                # hoist next chunk's deinterleave to the front of Pool's
                # per-chunk queue (its subs below wait on DVE anyway)
                deint(ci + 1)
                # slot 14 = min4 on DVE
                nc.vector.tensor_tensor(
                    wslot(14), grp(pmin[:, 0]), grp(pmin[:, 1]), ALU.min
                )
                # slot 15 = -min4 on Pool (tensor_scalar mult)
                nc.gpsimd.tensor_scalar(
                    wslot(15), wslot(14), -1.0, None, ALU.mult
                )

                # 14 slot subtractions as 7 double-width ops on Pool (GPSIMD
                # supports subtract; every operand ascending-stride by
                # construction)
                def wpair(s):
                    return w[:, :, s * 8:(s + 2) * 8].rearrange(
                        "p g (s e) -> p s g e", s=2
                    )

                def pl2(tns, a):
                    return tns[:, a:a + 2].rearrange(
                        "p s (g e) -> p s g e", e=8
                    )

                for s0, a_t, a_i, b_t, b_i in (
                    (0, xj, 0, tmax, 0),    # singles {0},{1}
                    (2, xj, 2, tmax, 2),    # singles {2},{3}
                    (4, pmin, 0, pmax, 0),  # pairs {0,1},{2,3}
                    (6, pmin, 2, pmax, 2),  # pairs {0,2},{1,3}
                    (8, pmin, 4, pmax, 4),  # pairs {0,3},{1,2}
                    (10, tmin, 0, xj, 0),   # triples excl 0, excl 1
                    (12, tmin, 2, xj, 2),   # triples excl 2, excl 3
                ):
                    nc.gpsimd.tensor_tensor(
                        wpair(s0), pl2(a_t, a_i), pl2(b_t, b_i),
                        ALU.subtract,
                    )

                # next chunk's DVE min side, ahead of this chunk's relu-evac
                # rounds in the DVE queue
                emit_mins(ci + 1)

                # contraction: per 16 groups of 8 i-rows: 16 transposes fill
                # a 2-bank PSUM tile; one relu-evac (ACT or DVE); 16 matmuls
                # fill a 2-bank psO tile; ACT cast-copies fp32 -> bf16; SP
                # DMAs out.
                for gg in range(0, ilen // 8, 16):
                    it = (t * I + ic0 + 8 * gg) // 128  # global round
                    pT = psT_pool.tile([128, 16, 128], wdt)
                    for u in range(16):
                        nc.tensor.transpose(pT[:, u], w[:, gg + u], ident)
                    lh = lh_pool.tile([128, 16, 128], wdt)
                    # first rounds -> ACT (DVE owns the first lattice); tail
                    # all-DVE (its lattice is done by then); steady state
                    # 2/5 on DVE, phase-tuned -> ~32/64 total on DVE
                    if it < 4:
                        wevac_dve = False
                    elif it >= 52:
                        wevac_dve = True
                    else:
                        wevac_dve = (it + 1) % 5 < 2
                    if wevac_dve:
                        nc.vector.tensor_scalar(
                            lh.rearrange("p a b -> p (a b)"),
                            pT.rearrange("p a b -> p (a b)"),
                            0.0,
                            None,
                            ALU.max,
                        )
                    else:
                        nc.scalar.activation(
                            lh.rearrange("p a b -> p (a b)"),
                            pT.rearrange("p a b -> p (a b)"),
                            AF.Relu,
                        )
                    pO = psO_pool.tile([128, 16, 64], f32)
                    for u in range(16):
                        g = gg + u          # local group in this chunk
                        gG = ic0 // 8 + g   # global group
                        nc.tensor.matmul(
                            pO[:, u],
                            lhsT=lh[:, u],
                            rhs=pd_sb[:, gG * 64:(gG + 1) * 64],
                            start=True,
                            stop=True,
                        )
                    ot = ot_pool.tile([128, 16, 64], wdt)
                    i0 = ic0 + gg * 8
                    if it == 63:
                        # final round: split the cast-copy + OUT DMA in half
                        # so the last DMA's fixed DGE latency overlaps the
                        # second half's copy
                        for h0 in (0, 8):
                            nc.scalar.activation(
                                ot[:, h0:h0 + 8].rearrange("p a b -> p (a b)"),
                                pO[:, h0:h0 + 8].rearrange("p a b -> p (a b)"),
                                AF.Copy,
                            )
                            nc.sync.dma_start(
                                out=OUTp[bsl, i0 + h0 * 8:i0 + h0 * 8 + 64, :],
                                in_=ot[:, h0:h0 + 8].rearrange(
                                    "p g (i o) -> p (g i) o", o=8
                                ),
                            )
                    else:
                        nc.scalar.activation(
                            ot.rearrange("p a b -> p (a b)"),
                            pO.rearrange("p a b -> p (a b)"),
                            AF.Copy,
                        )
                        nc.sync.dma_start(
                            out=OUTp[bsl, i0:i0 + 128, :],
                            in_=ot.rearrange("p g (i o) -> p (g i) o", o=8),
                        )
    if not nc.is_finalized():
        nc.finalize()
    return nc


def _get_nc():
    if "nc" not in _CACHE:
        _CACHE["nc"] = _build_bass()
    return _CACHE["nc"]


def kernel(X: np.ndarray, params: np.ndarray) -> np.ndarray:
    from concourse.bass_utils import run_bass_kernel_spmd

    X = np.asarray(X, dtype=np.float32).astype(ml_dtypes.bfloat16)
    params = np.asarray(params, dtype=np.float32)
    PD = _build_pd(params)

    nc = _get_nc()
    in_maps = [
        {"X": X[c * BC:(c + 1) * BC], "PD": PD} for c in range(NCORES)
    ]
    res = run_bass_kernel_spmd(nc, in_maps, list(range(NCORES)))
    out = np.concatenate(
        [np.asarray(res.results[c]["OUT"]) for c in range(NCORES)], axis=0
    )
    return out.astype(np.float32)



# revision 2
# speedup vs baseline: 1.4553x; 1.4553x over previous
"""Trainium2 Bass kernel for nn_BatchHighOrderActivation.

Math (Mobius rewrite of the Lovasz extension): the reference computes
    out[b,i,:] = sum_k coef_k * params[i, idx_k, :]
with sorted-diff coefs and nested top-set masks idx_k.  Since the Lovasz
extension is linear in the table F(m) = params[i,m,:], expand F over the
subset-indicator basis: F(m) = sum_{S subseteq m} a_S  (a = Mobius
transform of F).  The Lovasz extension of [S subseteq m] is min_{j in S} x_j,
so
    out[b,i,:] = sum_{S != 0} A[i,S,:] * min_{j in S} x_j
(params row 0 never appears in the reference output, so F(0) := 0 makes
the empty-set term vanish).  A is computed once on the host.

This kills the sort/subtract/relu pipeline entirely: per (b,i) only the 15
subset-minima are needed, 4 of which are x itself and 11 of which come from
6 double-width DVE min ops (pmin pairs -> triple mins -> min4).

Device layout per core (batch-sharded 8 ways, 1024 b-rows each):
  - i on PARTITIONS, b on free dim.  Per i-tile (128 i):
    W [128 i, 15 s, 1024 b] fp16; X DMAs straight into slots 0..3.
  - DVE: 6 min ops (all 2x fp16 packed mode) fill slots 4..14.
  - The layout crossing to matmul operands is a partition-shuffle DMA
    (not a PE transpose): per 8-i group,
        lhsT[q = i_sub*15+s, b] <- W[g*8+i_sub, s, b]
    one SBUF->SBUF DMA whose src (partition-major) and dest iteration
    orders match elementwise; 2KB/partition at full DMA rate.
  - PE: per group one matmul, K=120 (block-diag A table), M=128 b, N=64.
  - ACT/DVE: PSUM fp32 -> SBUF fp16 cast-evac; SP DMAs OUT (host upcasts).

Engine budget per core (v1 cost model, per 1/8-core "tile" ~ 9.2us):
SP queue: X 3.2 + OUT 6.3; Pool queue: 11 rearrange DMAs 8.7;
ACT: 5 rearrange DMAs 4.0 + evac share; DVE: lattice 6.2 + evac share;
PE: matmuls 3.4.
"""

import sys

for _p in ("/opt/trn_rl_repo", "/root/.axon_site/_ro/trn_rl_repo"):
    if _p not in sys.path:
        sys.path.append(_p)

import numpy as np

B, I, A, O = 8192, 1024, 4, 8
NCORES = 8
BC = B // NCORES          # batch rows per core
NS = 15                   # slots: 4 singles, 6 pairs, 4 triples, 1 full
QP = 8 * NS               # contraction partitions: q = i_sub*15 + s
NT = I // 128             # i-tiles per core
NGT = 16                  # groups of 8 i per i-tile
NG = NT * NGT             # global groups

# slot -> subset mask (bit j set iff coordinate j in S)
SLOT_MASKS = [
    1, 2, 4, 8,            # singles {0} {1} {2} {3}  (= x planes, DMA'd)
    3, 12, 5, 10, 9, 6,    # pairs {01} {23} {02} {13} {03} {12}
    14, 13, 11, 7,         # triples ~0 ~1 ~2 ~3
    15,                    # full set (min4)
]

_CACHE = {}


def _build_ad(params: np.ndarray) -> np.ndarray:
    """Block-diagonal Mobius table AD[q = i_sub*15 + s, G*64 + i_sub*8 + o]."""
    a = params.astype(np.float32).copy()
    a[:, 0, :] = 0.0                       # F(empty) := 0
    for bit in range(4):
        hi = np.where((np.arange(16) >> bit) & 1 == 1)[0]
        a[:, hi, :] = a[:, hi, :] - a[:, hi ^ (1 << bit), :]

    AD = np.zeros((QP, NG, 8, O), np.float32)
    for s, m in enumerate(SLOT_MASKS):
        for i_sub in range(8):
            AD[i_sub * NS + s, :, i_sub, :] = a[
                np.arange(NG) * 8 + i_sub, m, :
            ]
    return AD.reshape(QP, NG * 8 * O).astype(np.float16)


def _build_bass():
    import concourse.bass as bass
    import concourse.mybir as mybir
    import concourse.tile as tile
    from concourse import bacc

    f16 = mybir.dt.float16
    f32 = mybir.dt.float32
    ALU = mybir.AluOpType
    AF = mybir.ActivationFunctionType

    nc = bacc.Bacc(None)
    # X pre-transposed on host to [i, j, b] per core
    Xp = nc.declare_dram_parameter("X", [I, 4, BC], f16, isOutput=False)
    ADp = nc.declare_dram_parameter("AD", [QP, NG * 64], f16, isOutput=False)
    OUTp = nc.declare_dram_parameter("OUT", [BC, I, O], f16, isOutput=True)

    NB = BC // 128            # b-blocks per i-tile

    with tile.TileContext(nc) as tc:
        with (
            tc.tile_pool(name="consts", bufs=1) as consts,
            tc.tile_pool(name="w", bufs=3) as w_pool,
            tc.tile_pool(name="lh", bufs=2) as lh_pool,
            tc.tile_pool(name="ot", bufs=6) as ot_pool,
            tc.tile_pool(name="ps", bufs=3, space="PSUM") as ps_pool,
        ):
            ad_sb = consts.tile([QP, NG * 64], f16)
            # AD preload split across the three DMA queues so no single
            # queue eats the whole 16KB/p before the first X tile
            nc.scalar.dma_start(out=ad_sb[:, 0 * 2048:3 * 2048],
                                in_=ADp[:, 0 * 2048:3 * 2048])
            nc.gpsimd.dma_start(out=ad_sb[:, 3 * 2048:4 * 2048],
                                in_=ADp[:, 3 * 2048:4 * 2048])

            def lattice(w, bsl, blen):
                """Fill slots 4..14 of w[:, :, bsl] with the min lattice."""
                # (pmin01, pmin23) <- min((x0,x2), (x1,x3))
                nc.vector.tensor_tensor(
                    w[:, 4:6, bsl], w[:, 0:3:2, bsl], w[:, 1:4:2, bsl],
                    ALU.min)
                # (pmin02, pmin13) <- min((x0,x1), (x2,x3))
                nc.vector.tensor_tensor(
                    w[:, 6:8, bsl], w[:, 0:2, bsl], w[:, 2:4, bsl], ALU.min)
                # (pmin03, pmin12) <- min((x0,x1), (x3,x2))
                nc.vector.tensor_tensor(
                    w[:, 8:10, bsl], w[:, 0:2, bsl], w[:, 3:1:-1, bsl],
                    ALU.min)
                # (t~0, t~1) <- min(pmin23, (x1, x0))
                nc.vector.tensor_tensor(
                    w[:, 10:12, bsl],
                    w[:, 5:6, bsl].broadcast_to([128, 2, blen]),
                    w[:, 1::-1, bsl], ALU.min)
                # (t~2, t~3) <- min(pmin01, (x3, x2))
                nc.vector.tensor_tensor(
                    w[:, 12:14, bsl],
                    w[:, 4:5, bsl].broadcast_to([128, 2, blen]),
                    w[:, 3:1:-1, bsl], ALU.min)
                # min4 <- min(pmin01, pmin23)
                nc.vector.tensor_tensor(
                    w[:, 14, bsl], w[:, 4, bsl], w[:, 5, bsl], ALU.min)

            for t in range(NT):
                isl = slice(t * 128, (t + 1) * 128)
                w = w_pool.tile([128, NS, BC], f16)
                # X -> slots 0..3; halved so tile 0's lattice starts early
                nc.sync.dma_start(out=w[:, 0:4, 0:BC // 2],
                                  in_=Xp[isl, :, 0:BC // 2])
                nc.sync.dma_start(out=w[:, 0:4, BC // 2:],
                                  in_=Xp[isl, :, BC // 2:])
                if t == 0:
                    lattice(w, slice(0, BC // 2), BC // 2)
                    lattice(w, slice(BC // 2, BC), BC // 2)
                else:
                    lattice(w, slice(0, BC), BC)

                # partition-shuffle rearrange: per 8-i group one SBUF->SBUF
                # DMA; src iterates (i_sub, s, b) partition-major == dest
                # (q=(i_sub,s), b).  Pool carries most, ACT the rest.
                lh = lh_pool.tile([QP, NGT, BC], f16)
                for g in range(NGT):
                    q = nc.gpsimd if g < 11 else nc.scalar
                    q.dma_start(out=lh[:, g, :],
                                in_=w[g * 8:(g + 1) * 8, :, :])

                for bb in range(NB):
                    bsl = slice(bb * 128, (bb + 1) * 128)
                    pO = ps_pool.tile([128, NGT, 64], f32)
                    for g in range(NGT):
                        G = t * NGT + g
                        nc.tensor.matmul(
                            pO[:, g, :],
                            lhsT=lh[:, g, bsl],
                            rhs=ad_sb[:, G * 64:(G + 1) * 64],
                            start=True, stop=True)
                    ot = ot_pool.tile([128, NGT, 64], f16)
                    # PSUM fp32 -> SBUF fp16 cast; split ACT/DVE to balance
                    if bb % 4 < 3:
                        nc.scalar.activation(
                            ot.rearrange("p a b -> p (a b)"),
                            pO.rearrange("p a b -> p (a b)"), AF.Copy)
                    else:
                        nc.vector.tensor_scalar(
                            ot.rearrange("p a b -> p (a b)"),
                            pO.rearrange("p a b -> p (a b)"),
                            0.0, None, ALU.add)
                    nc.sync.dma_start(
                        out=OUTp[bsl, isl, :],
                        in_=ot.rearrange("p g (i o) -> p (g i) o", o=O))
    if not nc.is_finalized():
        nc.finalize()
    return nc


def _get_nc():
    if "nc" not in _CACHE:
        _CACHE["nc"] = _build_bass()
    return _CACHE["nc"]


def kernel(X: np.ndarray, params: np.ndarray) -> np.ndarray:
    from concourse.bass_utils import run_bass_kernel_spmd

    X = np.asarray(X, dtype=np.float32)
    params = np.asarray(params, dtype=np.float32)
    AD = _build_ad(params)

    nc = _get_nc()
    in_maps = []
    for c in range(NCORES):
        Xc = np.ascontiguousarray(
            X[c * BC:(c + 1) * BC].transpose(1, 2, 0)
        ).astype(np.float16)
        in_maps.append({"X": Xc, "AD": AD})
    res = run_bass_kernel_spmd(nc, in_maps, list(range(NCORES)))
    out = np.concatenate(
        [np.asarray(res.results[c]["OUT"]) for c in range(NCORES)], axis=0
    )
    return out.astype(np.float32)


# revision 5
# speedup vs baseline: 1.5037x; 1.0333x over previous
"""Trainium2 Bass kernel for nn_BatchHighOrderActivation.

Math (Mobius rewrite of the Lovasz extension): the reference computes
    out[b,i,:] = sum_k coef_k * params[i, idx_k, :]
with sorted-diff coefs and nested top-set masks idx_k.  Since the Lovasz
extension is linear in the table F(m) = params[i,m,:], expand F over the
subset-indicator basis: F(m) = sum_{S subseteq m} a_S  (a = Mobius
transform of F).  The Lovasz extension of [S subseteq m] is min_{j in S} x_j,
so
    out[b,i,:] = sum_{S != 0} A[i,S,:] * min_{j in S} x_j
(params row 0 never appears in the reference output, so F(0) := 0 makes
the empty-set term vanish).  A is computed once on the host.

This kills the sort/subtract/relu pipeline entirely: per (b,i) only the 15
subset-minima are needed, 4 of which are x itself and 11 of which come from
6 double-width DVE min ops (pmin pairs -> triple mins -> min4).

Device layout per core (batch-sharded 8 ways, 1024 b-rows each):
  - i on PARTITIONS, b on free dim.  Per i-tile (128 i):
    W [128 i, 15 s, 1024 b] fp16; X DMAs straight into slots 0..3.
  - DVE: 6 min ops (all 2x fp16 packed mode) fill slots 4..14.
  - The layout crossing to matmul operands is a partition-shuffle DMA
    (not a PE transpose): per 8-i group,
        lhsT[q = i_sub*15+s, b] <- W[g*8+i_sub, s, b]
    one SBUF->SBUF DMA whose src (partition-major) and dest iteration
    orders match elementwise; 2KB/partition at full DMA rate.
  - PE: per group one matmul, K=120 (block-diag A table), M=128 b, N=64.
  - ACT/DVE: PSUM fp32 -> SBUF fp16 cast-evac; SP DMAs OUT (host upcasts).

Engine budget per core (v1 cost model, per 1/8-core "tile" ~ 9.2us):
SP queue: X 3.2 + OUT 6.3; Pool queue: 11 rearrange DMAs 8.7;
ACT: 5 rearrange DMAs 4.0 + evac share; DVE: lattice 6.2 + evac share;
PE: matmuls 3.4.
"""

import sys

for _p in ("/opt/trn_rl_repo", "/root/.axon_site/_ro/trn_rl_repo"):
    if _p not in sys.path:
        sys.path.append(_p)

import numpy as np

B, I, A, O = 8192, 1024, 4, 8
NCORES = 8
BC = B // NCORES          # batch rows per core
NS = 15                   # slots: 4 singles, 6 pairs, 4 triples, 1 full
QP = 8 * NS               # contraction partitions: q = i_sub*15 + s
NT = I // 128             # i-tiles per core
NGT = 16                  # groups of 8 i per i-tile
NG = NT * NGT             # global groups

# slot -> subset mask (bit j set iff coordinate j in S)
SLOT_MASKS = [
    1, 2, 4, 8,            # singles {0} {1} {2} {3}  (= x planes, DMA'd)
    3, 12, 5, 10, 9, 6,    # pairs {01} {23} {02} {13} {03} {12}
    14, 13, 11, 7,         # triples ~0 ~1 ~2 ~3
    15,                    # full set (min4)
]

_CACHE = {}


def _build_ad(params: np.ndarray) -> np.ndarray:
    """Block-diagonal Mobius table AD[q = i_sub*15 + s, G*64 + i_sub*8 + o]."""
    a = params.astype(np.float32).copy()
    a[:, 0, :] = 0.0                       # F(empty) := 0
    for bit in range(4):
        hi = np.where((np.arange(16) >> bit) & 1 == 1)[0]
        a[:, hi, :] = a[:, hi, :] - a[:, hi ^ (1 << bit), :]

    AD = np.zeros((QP, NG, 8, O), np.float32)
    for s, m in enumerate(SLOT_MASKS):
        for i_sub in range(8):
            AD[i_sub * NS + s, :, i_sub, :] = a[
                np.arange(NG) * 8 + i_sub, m, :
            ]
    return AD.reshape(QP, NG * 8 * O).astype(np.float16)


def _build_bass():
    import concourse.bass as bass
    import concourse.mybir as mybir
    import concourse.tile as tile
    from concourse import bacc

    f16 = mybir.dt.float16
    f32 = mybir.dt.float32
    ALU = mybir.AluOpType
    AF = mybir.ActivationFunctionType

    nc = bacc.Bacc(None)
    # X pre-transposed on host to [i, j, b] per core
    Xp = nc.declare_dram_parameter("X", [I, 4, BC], f16, isOutput=False)
    ADp = nc.declare_dram_parameter("AD", [QP, NG * 64], f16, isOutput=False)
    OUTp = nc.declare_dram_parameter("OUT", [BC, I, O], f16, isOutput=True)

    NB = BC // 128            # b-blocks per i-tile

    with tile.TileContext(nc) as tc:
        with (
            tc.tile_pool(name="consts", bufs=1) as consts,
            tc.tile_pool(name="w", bufs=3) as w_pool,
            tc.tile_pool(name="lh", bufs=2) as lh_pool,
            tc.tile_pool(name="ot", bufs=4) as ot_pool,
            tc.tile_pool(name="ps", bufs=2, space="PSUM") as ps_pool,
        ):
            ad_sb = consts.tile([QP, NG * 64], f16)
            # AD loaded in per-i-tile slices, one tile ahead of its matmuls
            AD_CHUNK = NGT * 64

            def load_ad(t):
                if t >= NT:
                    return
                nc.scalar.dma_start(
                    out=ad_sb[:, t * AD_CHUNK:(t + 1) * AD_CHUNK],
                    in_=ADp[:, t * AD_CHUNK:(t + 1) * AD_CHUNK])

            load_ad(0)
            load_ad(1)

            def lattice(w, bsl, blen):
                """Fill slots 4..14 of w[:, :, bsl] with the min lattice."""
                # (pmin01, pmin23) <- min((x0,x2), (x1,x3))
                nc.vector.tensor_tensor(
                    w[:, 4:6, bsl], w[:, 0:3:2, bsl], w[:, 1:4:2, bsl],
                    ALU.min)
                # (pmin02, pmin13) <- min((x0,x1), (x2,x3))
                nc.vector.tensor_tensor(
                    w[:, 6:8, bsl], w[:, 0:2, bsl], w[:, 2:4, bsl], ALU.min)
                # (pmin03, pmin12) <- min((x0,x1), (x3,x2))
                nc.vector.tensor_tensor(
                    w[:, 8:10, bsl], w[:, 0:2, bsl], w[:, 3:1:-1, bsl],
                    ALU.min)
                # (t~0, t~1) <- min(pmin23, (x1, x0))
                nc.vector.tensor_tensor(
                    w[:, 10:12, bsl],
                    w[:, 5:6, bsl].broadcast_to([128, 2, blen]),
                    w[:, 1::-1, bsl], ALU.min)
                # (t~2, t~3) <- min(pmin01, (x3, x2))
                nc.vector.tensor_tensor(
                    w[:, 12:14, bsl],
                    w[:, 4:5, bsl].broadcast_to([128, 2, blen]),
                    w[:, 3:1:-1, bsl], ALU.min)
                # min4 <- min(pmin01, pmin23)
                nc.vector.tensor_tensor(
                    w[:, 14, bsl], w[:, 4, bsl], w[:, 5, bsl], ALU.min)

            for t in range(NT):
                isl = slice(t * 128, (t + 1) * 128)
                w = w_pool.tile([128, NS, BC], f16)
                # X -> slots 0..3; halved so tile 0's lattice starts early
                nc.sync.dma_start(out=w[:, 0:4, 0:BC // 2],
                                  in_=Xp[isl, :, 0:BC // 2])
                nc.sync.dma_start(out=w[:, 0:4, BC // 2:],
                                  in_=Xp[isl, :, BC // 2:])
                if t == 0:
                    lattice(w, slice(0, BC // 2), BC // 2)
                    lattice(w, slice(BC // 2, BC), BC // 2)
                else:
                    lattice(w, slice(0, BC), BC)

                if t == 1:
                    load_ad(2)

                # partition-shuffle rearrange: per 8-i group one SBUF->SBUF
                # DMA; src iterates (i_sub, s, b) partition-major == dest
                # (q=(i_sub,s), b).  Pool carries ~11.5, ACT ~4.5 per tile.
                lh = lh_pool.tile([QP, NGT, BC], f16)
                n_act = 4 if t % 2 else 5
                for g in range(NGT):
                    q = nc.scalar if g >= NGT - n_act else nc.gpsimd
                    q.dma_start(out=lh[:, g, :],
                                in_=w[g * 8:(g + 1) * 8, :, :])
                if t >= 2:
                    load_ad(t + 1)

                # two b-blocks per PSUM tile: 32 matmuls -> one 2048-wide
                # evac (amortizes the ACT/DVE access bubble) -> 2 OUT DMAs
                for bb2 in range(NB // 2):
                    pO = ps_pool.tile([128, 2, NGT, 64], f32)
                    for h in range(2):
                        bsl = slice((bb2 * 2 + h) * 128, (bb2 * 2 + h + 1) * 128)
                        for g in range(NGT):
                            G = t * NGT + g
                            nc.tensor.matmul(
                                pO[:, h, g, :],
                                lhsT=lh[:, g, bsl],
                                rhs=ad_sb[:, G * 64:(G + 1) * 64],
                                start=True, stop=True)
                    ot = ot_pool.tile([128, 2, NGT, 64], f16)
                    # PSUM fp32 -> SBUF fp16 cast; ~11/32 on DVE, rest ACT
                    r = t * (NB // 2) + bb2
                    if r % 3 == 1:
                        nc.vector.tensor_scalar(
                            ot.rearrange("p a g b -> p (a g b)"),
                            pO.rearrange("p a g b -> p (a g b)"),
                            0.0, None, ALU.add)
                    else:
                        nc.scalar.activation(
                            ot.rearrange("p a g b -> p (a g b)"),
                            pO.rearrange("p a g b -> p (a g b)"), AF.Copy)
                    for h in range(2):
                        bsl = slice((bb2 * 2 + h) * 128, (bb2 * 2 + h + 1) * 128)
                        nc.sync.dma_start(
                            out=OUTp[bsl, isl, :],
                            in_=ot[:, h].rearrange(
                                "p g (i o) -> p (g i) o", o=O))
    if not nc.is_finalized():
        nc.finalize()
    return nc


def _get_nc():
    if "nc" not in _CACHE:
        _CACHE["nc"] = _build_bass()
    return _CACHE["nc"]


def kernel(X: np.ndarray, params: np.ndarray) -> np.ndarray:
    from concourse.bass_utils import run_bass_kernel_spmd

    X = np.asarray(X, dtype=np.float32)
    params = np.asarray(params, dtype=np.float32)
    AD = _build_ad(params)

    nc = _get_nc()
    in_maps = []
    for c in range(NCORES):
        Xc = np.ascontiguousarray(
            X[c * BC:(c + 1) * BC].transpose(1, 2, 0)
        ).astype(np.float16)
        in_maps.append({"X": Xc, "AD": AD})
    res = run_bass_kernel_spmd(nc, in_maps, list(range(NCORES)))
    out = np.concatenate(
        [np.asarray(res.results[c]["OUT"]) for c in range(NCORES)], axis=0
    )
    return out.astype(np.float32)


# revision 6
# speedup vs baseline: 1.5631x; 1.0395x over previous
"""Trainium2 Bass kernel for nn_BatchHighOrderActivation.

Math (Mobius rewrite of the Lovasz extension): the reference computes
    out[b,i,:] = sum_k coef_k * params[i, idx_k, :]
with sorted-diff coefs and nested top-set masks idx_k.  Since the Lovasz
extension is linear in the table F(m) = params[i,m,:], expand F over the
subset-indicator basis: F(m) = sum_{S subseteq m} a_S  (a = Mobius
transform of F).  The Lovasz extension of [S subseteq m] is min_{j in S} x_j,
so
    out[b,i,:] = sum_{S != 0} A[i,S,:] * min_{j in S} x_j
(params row 0 never appears in the reference output, so F(0) := 0 makes
the empty-set term vanish).  A is computed once on the host.

This kills the sort/subtract/relu pipeline entirely: per (b,i) only the 15
subset-minima are needed, 4 of which are x itself and 11 of which come from
6 double-width DVE min ops (pmin pairs -> triple mins -> min4).

Device layout per core (batch-sharded 8 ways, 1024 b-rows each):
  - i on PARTITIONS, b on free dim.  Per i-tile (128 i):
    W [128 i, 15 s, 1024 b] fp16; X DMAs straight into slots 0..3.
  - DVE: 6 min ops (all 2x fp16 packed mode) fill slots 4..14.
  - The layout crossing to matmul operands is a partition-shuffle DMA
    (not a PE transpose): per 8-i group,
        lhsT[q = i_sub*15+s, b] <- W[g*8+i_sub, s, b]
    one SBUF->SBUF DMA whose src (partition-major) and dest iteration
    orders match elementwise; 2KB/partition at full DMA rate.
  - PE: per group one matmul, K=120 (block-diag A table), M=128 b, N=64.
  - ACT/DVE: PSUM fp32 -> SBUF fp16 cast-evac; SP DMAs OUT (host upcasts).

Engine budget per core (v1 cost model, per 1/8-core "tile" ~ 9.2us):
SP queue: X 3.2 + OUT 6.3; Pool queue: 11 rearrange DMAs 8.7;
ACT: 5 rearrange DMAs 4.0 + evac share; DVE: lattice 6.2 + evac share;
PE: matmuls 3.4.
"""

import sys

for _p in ("/opt/trn_rl_repo", "/root/.axon_site/_ro/trn_rl_repo"):
    if _p not in sys.path:
        sys.path.append(_p)

import numpy as np

B, I, A, O = 8192, 1024, 4, 8
NCORES = 8
BC = B // NCORES          # batch rows per core
NS = 15                   # slots: 4 singles, 6 pairs, 4 triples, 1 full
QP = 8 * NS               # contraction partitions: q = i_sub*15 + s
NT = I // 128             # i-tiles per core
NGT = 16                  # groups of 8 i per i-tile
NG = NT * NGT             # global groups

# slot -> subset mask (bit j set iff coordinate j in S)
SLOT_MASKS = [
    1, 2, 4, 8,            # singles {0} {1} {2} {3}  (= x planes, DMA'd)
    3, 12, 5, 10, 9, 6,    # pairs {01} {23} {02} {13} {03} {12}
    14, 13, 11, 7,         # triples ~0 ~1 ~2 ~3
    15,                    # full set (min4)
]

_CACHE = {}


def _build_ad(params: np.ndarray) -> np.ndarray:
    """Block-diagonal Mobius table AD[q = i_sub*15 + s, G*64 + i_sub*8 + o]."""
    a = params.astype(np.float32).copy()
    a[:, 0, :] = 0.0                       # F(empty) := 0
    for bit in range(4):
        hi = np.where((np.arange(16) >> bit) & 1 == 1)[0]
        a[:, hi, :] = a[:, hi, :] - a[:, hi ^ (1 << bit), :]

    AD = np.zeros((QP, NG, 8, O), np.float32)
    for s, m in enumerate(SLOT_MASKS):
        for i_sub in range(8):
            AD[i_sub * NS + s, :, i_sub, :] = a[
                np.arange(NG) * 8 + i_sub, m, :
            ]
    return AD.reshape(QP, NG * 8 * O).astype(np.float16)


def _build_bass():
    import concourse.bass as bass
    import concourse.mybir as mybir
    import concourse.tile as tile
    from concourse import bacc

    f16 = mybir.dt.float16
    f32 = mybir.dt.float32
    ALU = mybir.AluOpType
    AF = mybir.ActivationFunctionType

    nc = bacc.Bacc(None)
    # X pre-transposed on host to [i, j, b] per core
    Xp = nc.declare_dram_parameter("X", [I, 4, BC], f16, isOutput=False)
    ADp = nc.declare_dram_parameter("AD", [QP, NG * 64], f16, isOutput=False)
    OUTp = nc.declare_dram_parameter("OUT", [BC, I, O], f16, isOutput=True)

    NB = BC // 128            # b-blocks per i-tile

    with tile.TileContext(nc) as tc:
        with (
            tc.tile_pool(name="consts", bufs=1) as consts,
            tc.tile_pool(name="w", bufs=3) as w_pool,
            tc.tile_pool(name="lh", bufs=2) as lh_pool,
            tc.tile_pool(name="ot", bufs=4) as ot_pool,
            tc.tile_pool(name="ps", bufs=2, space="PSUM") as ps_pool,
        ):
            ad_sb = consts.tile([QP, NG * 64], f16)
            # AD loaded in per-i-tile slices, one tile ahead of its matmuls
            AD_CHUNK = NGT * 64

            def load_ad(t):
                if t >= NT:
                    return
                nc.scalar.dma_start(
                    out=ad_sb[:, t * AD_CHUNK:(t + 1) * AD_CHUNK],
                    in_=ADp[:, t * AD_CHUNK:(t + 1) * AD_CHUNK])

            load_ad(0)
            load_ad(1)

            def lattice(w, bsl, blen):
                """Fill slots 4..14 of w[:, :, bsl] with the min lattice."""
                # (pmin01, pmin23) <- min((x0,x2), (x1,x3))
                nc.vector.tensor_tensor(
                    w[:, 4:6, bsl], w[:, 0:3:2, bsl], w[:, 1:4:2, bsl],
                    ALU.min)
                # (pmin02, pmin13) <- min((x0,x1), (x2,x3))
                nc.vector.tensor_tensor(
                    w[:, 6:8, bsl], w[:, 0:2, bsl], w[:, 2:4, bsl], ALU.min)
                # (pmin03, pmin12) <- min((x0,x1), (x3,x2))
                nc.vector.tensor_tensor(
                    w[:, 8:10, bsl], w[:, 0:2, bsl], w[:, 3:1:-1, bsl],
                    ALU.min)
                # (t~0, t~1) <- min(pmin23, (x1, x0))
                nc.vector.tensor_tensor(
                    w[:, 10:12, bsl],
                    w[:, 5:6, bsl].broadcast_to([128, 2, blen]),
                    w[:, 1::-1, bsl], ALU.min)
                # (t~2, t~3) <- min(pmin01, (x3, x2))
                nc.vector.tensor_tensor(
                    w[:, 12:14, bsl],
                    w[:, 4:5, bsl].broadcast_to([128, 2, blen]),
                    w[:, 3:1:-1, bsl], ALU.min)
                # min4 <- min(pmin01, pmin23)
                nc.vector.tensor_tensor(
                    w[:, 14, bsl], w[:, 4, bsl], w[:, 5, bsl], ALU.min)

            def contract(t, lh, b0, nblk, blk, evac_dve, out_qs):
                """Matmul+evac+OUT for b-range [b0, b0+nblk*128*blk) of
                i-tile t, in PSUM units of `blk` 128-b blocks."""
                isl = slice(t * 128, (t + 1) * 128)
                for u in range(nblk):
                    pO = ps_pool.tile([128, blk, NGT, 64], f32)
                    for h in range(blk):
                        bsl = slice(b0 + (u * blk + h) * 128,
                                    b0 + (u * blk + h + 1) * 128)
                        for g in range(NGT):
                            G = t * NGT + g
                            nc.tensor.matmul(
                                pO[:, h, g, :],
                                lhsT=lh[:, g, bsl],
                                rhs=ad_sb[:, G * 64:(G + 1) * 64],
                                start=True, stop=True)
                    ot = ot_pool.tile([128, blk, NGT, 64], f16)
                    if evac_dve(u):
                        nc.vector.tensor_scalar(
                            ot.rearrange("p a g b -> p (a g b)"),
                            pO.rearrange("p a g b -> p (a g b)"),
                            0.0, None, ALU.add)
                    else:
                        nc.scalar.activation(
                            ot.rearrange("p a g b -> p (a g b)"),
                            pO.rearrange("p a g b -> p (a g b)"), AF.Copy)
                    for h in range(blk):
                        bsl = slice(b0 + (u * blk + h) * 128,
                                    b0 + (u * blk + h + 1) * 128)
                        out_qs(u, h).dma_start(
                            out=OUTp[bsl, isl, :],
                            in_=ot[:, h].rearrange(
                                "p g (i o) -> p (g i) o", o=O))

            for t in range(NT):
                isl = slice(t * 128, (t + 1) * 128)
                w = w_pool.tile([128, NS, BC], f16)
                # X -> slots 0..3.  Tile 0 loads on Pool (idle at start) in
                # halves so its lattice starts early; later tiles on SP.
                xq = nc.gpsimd if t == 0 else nc.sync
                xq.dma_start(out=w[:, 0:4, 0:BC // 2],
                             in_=Xp[isl, :, 0:BC // 2])
                xq.dma_start(out=w[:, 0:4, BC // 2:],
                             in_=Xp[isl, :, BC // 2:])

                lh = lh_pool.tile([QP, NGT, BC], f16)
                if t == 0:
                    # two b=512 passes; rearranges spread over Pool/ACT/SP
                    for half in range(2):
                        hsl = slice(half * 512, half * 512 + 512)
                        lattice(w, hsl, 512)
                        for g in range(NGT):
                            q = (nc.gpsimd, nc.scalar, nc.scalar,
                                 nc.sync)[g % 4]
                            q.dma_start(out=lh[:, g, hsl],
                                        in_=w[g * 8:(g + 1) * 8, :, hsl])
                        contract(t, lh, half * 512, 2, 2,
                                 lambda u: u == 1,
                                 lambda u, h: nc.sync)
                    load_ad(2)
                    continue

                lattice(w, slice(0, BC), BC)
                # partition-shuffle rearrange: per 8-i group one SBUF->SBUF
                # DMA; src iterates (i_sub, s, b) partition-major == dest
                # (q=(i_sub,s), b).  Pool carries ~11.5, ACT ~4.5 per tile.
                n_act = 4 if t % 2 else 5
                for g in range(NGT):
                    q = nc.scalar if g >= NGT - n_act else nc.gpsimd
                    q.dma_start(out=lh[:, g, :],
                                in_=w[g * 8:(g + 1) * 8, :, :])
                load_ad(t + 1)

                if t < NT - 1:
                    # 2 b-blocks per PSUM unit: 32 matmuls -> one 2048-wide
                    # evac (amortized bubble) -> 2 OUT DMAs on SP
                    contract(t, lh, 0, NB // 2, 2,
                             lambda u, _t=t: (_t * 4 + u) % 3 == 1,
                             lambda u, h: nc.sync)
                else:
                    # drain tile: 1-block units, evacs alternate DVE/ACT,
                    # OUT DMAs spread across all three queues
                    contract(t, lh, 0, NB, 1,
                             lambda u: u % 2 == 1,
                             lambda u, h: (nc.sync, nc.sync, nc.sync,
                                           nc.sync, nc.scalar, nc.scalar,
                                           nc.gpsimd, nc.gpsimd)[u])
    if not nc.is_finalized():
        nc.finalize()
    return nc


def _get_nc():
    if "nc" not in _CACHE:
        _CACHE["nc"] = _build_bass()
    return _CACHE["nc"]


def kernel(X: np.ndarray, params: np.ndarray) -> np.ndarray:
    from concourse.bass_utils import run_bass_kernel_spmd

    X = np.asarray(X, dtype=np.float32)
    params = np.asarray(params, dtype=np.float32)
    AD = _build_ad(params)

    nc = _get_nc()
    in_maps = []
    for c in range(NCORES):
        Xc = np.ascontiguousarray(
            X[c * BC:(c + 1) * BC].transpose(1, 2, 0)
        ).astype(np.float16)
        in_maps.append({"X": Xc, "AD": AD})
    res = run_bass_kernel_spmd(nc, in_maps, list(range(NCORES)))
    out = np.concatenate(
        [np.asarray(res.results[c]["OUT"]) for c in range(NCORES)], axis=0
    )
    return out.astype(np.float32)


# revision 19
# speedup vs baseline: 1.6219x; 1.0376x over previous
"""Trainium2 Bass kernel for nn_BatchHighOrderActivation.

Math (Mobius rewrite of the Lovasz extension): the reference computes
    out[b,i,:] = sum_k coef_k * params[i, idx_k, :]
with sorted-diff coefs and nested top-set masks idx_k.  Since the Lovasz
extension is linear in the table F(m) = params[i,m,:], expand F over the
subset-indicator basis: F(m) = sum_{S subseteq m} a_S  (a = Mobius
transform of F).  The Lovasz extension of [S subseteq m] is min_{j in S} x_j,
so
    out[b,i,:] = sum_{S != 0} A[i,S,:] * min_{j in S} x_j
(params row 0 never appears in the reference output, so F(0) := 0 makes
the empty-set term vanish).  A is computed once on the host.

This kills the sort/subtract/relu pipeline entirely: per (b,i) only the 15
subset-minima are needed, 4 of which are x itself and 11 of which come from
6 double-width DVE min ops (pmin pairs -> triple mins -> min4).

Device layout per core (batch-sharded 8 ways, 1024 b-rows each):
  - i on PARTITIONS, b on free dim.  Per i-tile (128 i):
    W [128 i, 15 s, 1024 b] fp16; X DMAs straight into slots 0..3.
  - DVE: 6 min ops (all 2x fp16 packed mode) fill slots 4..14.
  - The layout crossing to matmul operands is a partition-shuffle DMA
    (not a PE transpose): per 8-i group,
        lhsT[q = i_sub*15+s, b] <- W[g*8+i_sub, s, b]
    one SBUF->SBUF DMA whose src (partition-major) and dest iteration
    orders match elementwise; 2KB/partition at full DMA rate.
  - PE: per group one matmul, K=120 (block-diag A table), M=128 b, N=64.
  - ACT/DVE: PSUM fp32 -> SBUF fp16 cast-evac; SP DMAs OUT (host upcasts).

Engine budget per core (v1 cost model, per 1/8-core "tile" ~ 9.2us):
SP queue: X 3.2 + OUT 6.3; Pool queue: 11 rearrange DMAs 8.7;
ACT: 5 rearrange DMAs 4.0 + evac share; DVE: lattice 6.2 + evac share;
PE: matmuls 3.4.
"""

import sys

for _p in ("/opt/trn_rl_repo", "/root/.axon_site/_ro/trn_rl_repo"):
    if _p not in sys.path:
        sys.path.append(_p)

import numpy as np

B, I, A, O = 8192, 1024, 4, 8
NCORES = 8
BC = B // NCORES          # batch rows per core
NS = 15                   # slots: 4 singles, 6 pairs, 4 triples, 1 full
QP = 8 * NS               # contraction partitions: q = i_sub*15 + s
NT = I // 128             # i-tiles per core
NGT = 16                  # groups of 8 i per i-tile
NG = NT * NGT             # global groups

# slot -> subset mask (bit j set iff coordinate j in S)
SLOT_MASKS = [
    1, 2, 4, 8,            # singles {0} {1} {2} {3}  (= x planes, DMA'd)
    3, 12, 5, 10, 9, 6,    # pairs {01} {23} {02} {13} {03} {12}
    14, 13, 11, 7,         # triples ~0 ~1 ~2 ~3
    15,                    # full set (min4)
]

_CACHE = {}


def _build_ad(params: np.ndarray) -> np.ndarray:
    """Block-diagonal Mobius table AD[q = i_sub*15 + s, G*64 + i_sub*8 + o]."""
    a = params.astype(np.float32).copy()
    a[:, 0, :] = 0.0                       # F(empty) := 0
    for bit in range(4):
        hi = np.where((np.arange(16) >> bit) & 1 == 1)[0]
        a[:, hi, :] = a[:, hi, :] - a[:, hi ^ (1 << bit), :]

    AD = np.zeros((QP, NG, 8, O), np.float32)
    for s, m in enumerate(SLOT_MASKS):
        for i_sub in range(8):
            AD[i_sub * NS + s, :, i_sub, :] = a[
                np.arange(NG) * 8 + i_sub, m, :
            ]
    return AD.reshape(QP, NG * 8 * O).astype(np.float16)


def _build_bass():
    import concourse.bass as bass
    import concourse.mybir as mybir
    import concourse.tile as tile
    from concourse import bacc

    f16 = mybir.dt.float16
    f32 = mybir.dt.float32
    ALU = mybir.AluOpType
    AF = mybir.ActivationFunctionType

    nc = bacc.Bacc(None)
    # X pre-transposed on host to [i, j, b] per core
    Xp = nc.declare_dram_parameter("X", [I, 4, BC], f16, isOutput=False)
    ADp = nc.declare_dram_parameter("AD", [QP, NG * 64], f16, isOutput=False)
    OUTp = nc.declare_dram_parameter("OUT", [BC, I, O], f16, isOutput=True)

    NB = BC // 128            # b-blocks per i-tile

    with tile.TileContext(nc) as tc:
        with (
            tc.tile_pool(name="consts", bufs=1) as consts,
            tc.tile_pool(name="w", bufs=3) as w_pool,
            tc.tile_pool(name="lh", bufs=2) as lh_pool,
            tc.tile_pool(name="ot", bufs=8) as ot_pool,
            tc.tile_pool(name="ps", bufs=4, space="PSUM") as ps_pool,
        ):
            ad_sb = consts.tile([QP, NG * 64], f16)
            # AD loaded in per-i-tile slices, one tile ahead of its matmuls
            AD_CHUNK = NGT * 64

            def load_ad(t):
                if t >= NT:
                    return
                nc.scalar.dma_start(
                    out=ad_sb[:, t * AD_CHUNK:(t + 1) * AD_CHUNK],
                    in_=ADp[:, t * AD_CHUNK:(t + 1) * AD_CHUNK])

            load_ad(0)
            load_ad(1)

            def lattice(w, bsl, blen):
                """Fill slots 4..14 of w[:, :, bsl] with the min lattice."""
                # (pmin01, pmin23) <- min((x0,x2), (x1,x3))
                nc.vector.tensor_tensor(
                    w[:, 4:6, bsl], w[:, 0:3:2, bsl], w[:, 1:4:2, bsl],
                    ALU.min)
                # (pmin02, pmin13) <- min((x0,x1), (x2,x3))
                nc.vector.tensor_tensor(
                    w[:, 6:8, bsl], w[:, 0:2, bsl], w[:, 2:4, bsl], ALU.min)
                # (pmin03, pmin12) <- min((x0,x1), (x3,x2))
                nc.vector.tensor_tensor(
                    w[:, 8:10, bsl], w[:, 0:2, bsl], w[:, 3:1:-1, bsl],
                    ALU.min)
                # (t~0, t~1) <- min(pmin23, (x1, x0))
                nc.vector.tensor_tensor(
                    w[:, 10:12, bsl],
                    w[:, 5:6, bsl].broadcast_to([128, 2, blen]),
                    w[:, 1::-1, bsl], ALU.min)
                # (t~2, t~3) <- min(pmin01, (x3, x2))
                nc.vector.tensor_tensor(
                    w[:, 12:14, bsl],
                    w[:, 4:5, bsl].broadcast_to([128, 2, blen]),
                    w[:, 3:1:-1, bsl], ALU.min)
                # min4 <- min(pmin01, pmin23)
                nc.vector.tensor_tensor(
                    w[:, 14, bsl], w[:, 4, bsl], w[:, 5, bsl], ALU.min)

            def contract(t, lh, b0, nblk, blk, evac_dve, out_qs):
                """Matmul+evac+OUT for b-range [b0, b0+nblk*128*blk) of
                i-tile t, in PSUM units of `blk` 128-b blocks."""
                isl = slice(t * 128, (t + 1) * 128)
                for u in range(nblk):
                    pO = ps_pool.tile([128, blk, NGT, 64], f32)
                    for h in range(blk):
                        bsl = slice(b0 + (u * blk + h) * 128,
                                    b0 + (u * blk + h + 1) * 128)
                        for g in range(NGT):
                            G = t * NGT + g
                            nc.tensor.matmul(
                                pO[:, h, g, :],
                                lhsT=lh[:, g, bsl],
                                rhs=ad_sb[:, G * 64:(G + 1) * 64],
                                start=True, stop=True)
                    ot = ot_pool.tile([128, blk, NGT, 64], f16)
                    if evac_dve(u):
                        nc.vector.tensor_scalar(
                            ot.rearrange("p a g b -> p (a g b)"),
                            pO.rearrange("p a g b -> p (a g b)"),
                            0.0, None, ALU.add)
                    else:
                        nc.scalar.activation(
                            ot.rearrange("p a g b -> p (a g b)"),
                            pO.rearrange("p a g b -> p (a g b)"), AF.Copy)
                    for h in range(blk):
                        bsl = slice(b0 + (u * blk + h) * 128,
                                    b0 + (u * blk + h + 1) * 128)
                        out_qs(u, h).dma_start(
                            out=OUTp[bsl, isl, :],
                            in_=ot[:, h].rearrange(
                                "p g (i o) -> p (g i) o", o=O))

            # staged emission: contract(t-1) is emitted AFTER tile t's
            # lattice+rearrange, so every engine's program order runs
            # next-tile feed work ahead of previous-tile drain work.
            prev_lh = None
            for t in range(NT):
                isl = slice(t * 128, (t + 1) * 128)
                w = w_pool.tile([128, NS, BC], f16)
                # X -> slots 0..3.  Tile 0 loads on Pool (idle at start) in
                # halves so its lattice starts early; later tiles on SP.
                xq = nc.gpsimd if t == 0 else nc.sync
                xq.dma_start(out=w[:, 0:4, 0:BC // 2],
                             in_=Xp[isl, :, 0:BC // 2])
                xq.dma_start(out=w[:, 0:4, BC // 2:],
                             in_=Xp[isl, :, BC // 2:])

                lh = lh_pool.tile([QP, NGT, BC], f16)
                if t == 0:
                    # two b=512 passes with immediate contracts so the OUT
                    # stream starts early; rearranges over Pool/ACT/SP
                    for half in range(2):
                        hsl = slice(half * 512, half * 512 + 512)
                        lattice(w, hsl, 512)
                        for g in range(NGT):
                            q = (nc.gpsimd, nc.scalar, nc.gpsimd,
                                 nc.sync)[g % 4]
                            q.dma_start(out=lh[:, g, hsl],
                                        in_=w[g * 8:(g + 1) * 8, :, hsl])
                        contract(t, lh, half * 512, 4, 1,
                                 lambda u: False,
                                 lambda u, h: nc.sync)
                elif t == NT - 1:
                    # drain tile: lattice first (DVE is the critical path),
                    # then contract(t-2) with DVE-heavy evacs (DVE is done
                    # after the lattice), tile-7 contracts with ACT evacs,
                    # OUT spread over all queues
                    lattice(w, slice(0, 512), 512)
                    for g in range(NGT):
                        q = (nc.gpsimd, nc.gpsimd, nc.gpsimd,
                             nc.sync)[g % 4]
                        q.dma_start(out=lh[:, g, 0:512],
                                    in_=w[g * 8:(g + 1) * 8, :, 0:512])
                    lattice(w, slice(512, BC), 512)
                    if prev_lh is not None:
                        contract(t - 1, prev_lh, 0, NB, 1,
                                 lambda u: u >= 2,
                                 lambda u, h: nc.sync)
                        prev_lh = None
                    for g in range(NGT):
                        q = (nc.gpsimd, nc.gpsimd, nc.gpsimd,
                             nc.sync)[g % 4]
                        q.dma_start(out=lh[:, g, 512:],
                                    in_=w[g * 8:(g + 1) * 8, :, 512:])
                    for half in range(2):
                        contract(t, lh, half * 512, 4, 1,
                                 lambda u: u % 2 == 1,
                                 lambda u, h: (nc.sync, nc.gpsimd,
                                               nc.sync, nc.scalar)[u])
                    continue
                else:
                    lattice(w, slice(0, BC), BC)
                    # partition-shuffle rearrange: per 8-i group one
                    # SBUF->SBUF DMA; src iterates (i_sub, s, b)
                    # partition-major == dest (q=(i_sub,s), b).
                    n_act = 5
                    for g in range(NGT):
                        if t == 1 and g < 2:
                            q = nc.sync          # SP has early slack
                        elif g >= NGT - n_act:
                            q = nc.scalar
                        else:
                            q = nc.gpsimd
                        q.dma_start(out=lh[:, g, :],
                                    in_=w[g * 8:(g + 1) * 8, :, :])
                if t >= 1:
                    load_ad(t + 1)

                if prev_lh is not None:
                    # 1-block PSUM units: 16 matmuls -> one 1024-wide
                    # evac -> one OUT DMA on SP
                    contract(t - 1, prev_lh, 0, NB, 1,
                             lambda u, _t=t - 1: u < (3 if _t % 2 else 2),
                             lambda u, h: nc.sync)
                prev_lh = None if t == 0 else lh
    if not nc.is_finalized():
        nc.finalize()
    return nc


def _get_nc():
    if "nc" not in _CACHE:
        _CACHE["nc"] = _build_bass()
    return _CACHE["nc"]


def kernel(X: np.ndarray, params: np.ndarray) -> np.ndarray:
    from concourse.bass_utils import run_bass_kernel_spmd

    X = np.asarray(X, dtype=np.float32)
    params = np.asarray(params, dtype=np.float32)
    AD = _build_ad(params)

    nc = _get_nc()
    in_maps = []
    for c in range(NCORES):
        Xc = np.ascontiguousarray(
            X[c * BC:(c + 1) * BC].transpose(1, 2, 0)
        ).astype(np.float16)
        in_maps.append({"X": Xc, "AD": AD})
    res = run_bass_kernel_spmd(nc, in_maps, list(range(NCORES)))
    out = np.concatenate(
        [np.asarray(res.results[c]["OUT"]) for c in range(NCORES)], axis=0
    )
    return out.astype(np.float32)


# revision 26
# speedup vs baseline: 1.6362x; 1.0088x over previous
"""Trainium2 Bass kernel for nn_BatchHighOrderActivation.

Math (Mobius rewrite of the Lovasz extension): the reference computes
    out[b,i,:] = sum_k coef_k * params[i, idx_k, :]
with sorted-diff coefs and nested top-set masks idx_k.  Since the Lovasz
extension is linear in the table F(m) = params[i,m,:], expand F over the
subset-indicator basis: F(m) = sum_{S subseteq m} a_S  (a = Mobius
transform of F).  The Lovasz extension of [S subseteq m] is min_{j in S} x_j,
so
    out[b,i,:] = sum_{S != 0} A[i,S,:] * min_{j in S} x_j
(params row 0 never appears in the reference output, so F(0) := 0 makes
the empty-set term vanish).  A is computed once on the host.

This kills the sort/subtract/relu pipeline entirely: per (b,i) only the 15
subset-minima are needed, 4 of which are x itself and 11 of which come from
6 double-width DVE min ops (pmin pairs -> triple mins -> min4).

Device layout per core (batch-sharded 8 ways, 1024 b-rows each):
  - i on PARTITIONS, b on free dim.  Per i-tile (128 i):
    W [128 i, 15 s, 1024 b] fp16; X DMAs straight into slots 0..3.
  - DVE: 6 min ops (all 2x fp16 packed mode) fill slots 4..14.
  - The layout crossing to matmul operands is a partition-shuffle DMA
    (not a PE transpose): per 8-i group,
        lhsT[q = i_sub*15+s, b] <- W[g*8+i_sub, s, b]
    one SBUF->SBUF DMA whose src (partition-major) and dest iteration
    orders match elementwise; 2KB/partition at full DMA rate.
  - PE: per group one matmul, K=120 (block-diag A table), M=128 b, N=64.
  - ACT/DVE: PSUM fp32 -> SBUF fp16 cast-evac; SP DMAs OUT (host upcasts).

Engine budget per core (v1 cost model, per 1/8-core "tile" ~ 9.2us):
SP queue: X 3.2 + OUT 6.3; Pool queue: 11 rearrange DMAs 8.7;
ACT: 5 rearrange DMAs 4.0 + evac share; DVE: lattice 6.2 + evac share;
PE: matmuls 3.4.
"""

import sys

for _p in ("/opt/trn_rl_repo", "/root/.axon_site/_ro/trn_rl_repo"):
    if _p not in sys.path:
        sys.path.append(_p)

import numpy as np

B, I, A, O = 8192, 1024, 4, 8
NCORES = 8
BC = B // NCORES          # batch rows per core
NS = 15                   # slots: 4 singles, 6 pairs, 4 triples, 1 full
QP = 8 * NS               # contraction partitions: q = i_sub*15 + s
NT = I // 128             # i-tiles per core
NGT = 16                  # groups of 8 i per i-tile
NG = NT * NGT             # global groups

# slot -> subset mask (bit j set iff coordinate j in S)
SLOT_MASKS = [
    1, 2, 4, 8,            # singles {0} {1} {2} {3}  (= x planes, DMA'd)
    3, 12, 5, 10, 9, 6,    # pairs {01} {23} {02} {13} {03} {12}
    14, 13, 11, 7,         # triples ~0 ~1 ~2 ~3
    15,                    # full set (min4)
]

_CACHE = {}


def _build_ad(params: np.ndarray) -> np.ndarray:
    """Block-diagonal Mobius table AD[q = i_sub*15 + s, G*64 + i_sub*8 + o]."""
    a = params.astype(np.float32).copy()
    a[:, 0, :] = 0.0                       # F(empty) := 0
    for bit in range(4):
        hi = np.where((np.arange(16) >> bit) & 1 == 1)[0]
        a[:, hi, :] = a[:, hi, :] - a[:, hi ^ (1 << bit), :]

    AD = np.zeros((QP, NG, 8, O), np.float32)
    for s, m in enumerate(SLOT_MASKS):
        for i_sub in range(8):
            AD[i_sub * NS + s, :, i_sub, :] = a[
                np.arange(NG) * 8 + i_sub, m, :
            ]
    return AD.reshape(QP, NG * 8 * O).astype(np.float16)


def _build_bass():
    import concourse.bass as bass
    import concourse.mybir as mybir
    import concourse.tile as tile
    from concourse import bacc

    f16 = mybir.dt.float16
    f32 = mybir.dt.float32
    ALU = mybir.AluOpType
    AF = mybir.ActivationFunctionType

    nc = bacc.Bacc(None)
    # X pre-transposed on host to [i, j, b] per core
    Xp = nc.declare_dram_parameter("X", [I, 4, BC], f16, isOutput=False)
    ADp = nc.declare_dram_parameter("AD", [QP, NG * 64], f16, isOutput=False)
    OUTp = nc.declare_dram_parameter("OUT", [BC, I, O], f16, isOutput=True)

    NB = BC // 128            # b-blocks per i-tile

    with tile.TileContext(nc) as tc:
        with (
            tc.tile_pool(name="consts", bufs=1) as consts,
            tc.tile_pool(name="w", bufs=3) as w_pool,
            tc.tile_pool(name="lh", bufs=2) as lh_pool,
            tc.tile_pool(name="ot", bufs=8) as ot_pool,
            tc.tile_pool(name="ps", bufs=4, space="PSUM") as ps_pool,
        ):
            ad_sb = consts.tile([QP, NG * 64], f16)
            # AD loaded in per-i-tile slices, one tile ahead of its matmuls
            AD_CHUNK = NGT * 64

            def load_ad(t):
                if t >= NT:
                    return
                q = nc.scalar if t % 2 == 0 else nc.gpsimd
                q.dma_start(
                    out=ad_sb[:, t * AD_CHUNK:(t + 1) * AD_CHUNK],
                    in_=ADp[:, t * AD_CHUNK:(t + 1) * AD_CHUNK])

            load_ad(0)
            load_ad(1)

            def lattice(w, bsl, blen):
                """Fill slots 4..14 of w[:, :, bsl] with the min lattice."""
                # (pmin01, pmin23) <- min((x0,x2), (x1,x3))
                nc.vector.tensor_tensor(
                    w[:, 4:6, bsl], w[:, 0:3:2, bsl], w[:, 1:4:2, bsl],
                    ALU.min)
                # (pmin02, pmin13) <- min((x0,x1), (x2,x3))
                nc.vector.tensor_tensor(
                    w[:, 6:8, bsl], w[:, 0:2, bsl], w[:, 2:4, bsl], ALU.min)
                # (pmin03, pmin12) <- min((x0,x1), (x3,x2))
                nc.vector.tensor_tensor(
                    w[:, 8:10, bsl], w[:, 0:2, bsl], w[:, 3:1:-1, bsl],
                    ALU.min)
                # (t~0, t~1) <- min(pmin23, (x1, x0))
                nc.vector.tensor_tensor(
                    w[:, 10:12, bsl],
                    w[:, 5:6, bsl].broadcast_to([128, 2, blen]),
                    w[:, 1::-1, bsl], ALU.min)
                # (t~2, t~3) <- min(pmin01, (x3, x2))
                nc.vector.tensor_tensor(
                    w[:, 12:14, bsl],
                    w[:, 4:5, bsl].broadcast_to([128, 2, blen]),
                    w[:, 3:1:-1, bsl], ALU.min)
                # min4 <- min(pmin01, pmin23)
                nc.vector.tensor_tensor(
                    w[:, 14, bsl], w[:, 4, bsl], w[:, 5, bsl], ALU.min)

            def contract(t, lh, b0, nblk, blk, evac_dve, out_qs):
                """Matmul+evac+OUT for b-range [b0, b0+nblk*128*blk) of
                i-tile t, in PSUM units of `blk` 128-b blocks."""
                isl = slice(t * 128, (t + 1) * 128)
                for u in range(nblk):
                    pO = ps_pool.tile([128, blk, NGT, 64], f32)
                    for h in range(blk):
                        bsl = slice(b0 + (u * blk + h) * 128,
                                    b0 + (u * blk + h + 1) * 128)
                        for g in range(NGT):
                            G = t * NGT + g
                            nc.tensor.matmul(
                                pO[:, h, g, :],
                                lhsT=lh[:, g, bsl],
                                rhs=ad_sb[:, G * 64:(G + 1) * 64],
                                start=True, stop=True)
                    ot = ot_pool.tile([128, blk, NGT, 64], f16)
                    if evac_dve(u):
                        nc.vector.tensor_scalar(
                            ot.rearrange("p a g b -> p (a g b)"),
                            pO.rearrange("p a g b -> p (a g b)"),
                            0.0, None, ALU.add)
                    else:
                        nc.scalar.activation(
                            ot.rearrange("p a g b -> p (a g b)"),
                            pO.rearrange("p a g b -> p (a g b)"), AF.Copy)
                    for h in range(blk):
                        bsl = slice(b0 + (u * blk + h) * 128,
                                    b0 + (u * blk + h + 1) * 128)
                        out_qs(u, h).dma_start(
                            out=OUTp[bsl, isl, :],
                            in_=ot[:, h].rearrange(
                                "p g (i o) -> p (g i) o", o=O))

            # staged emission: contract(t-1) is emitted AFTER tile t's
            # lattice+rearrange, so every engine's program order runs
            # next-tile feed work ahead of previous-tile drain work.
            prev_lh = None
            for t in range(NT):
                isl = slice(t * 128, (t + 1) * 128)
                w = w_pool.tile([128, NS, BC], f16)
                # X -> slots 0..3.  Tile 0 loads on Pool (idle at start) in
                # halves so its lattice starts early; later tiles on SP.
                xq = nc.gpsimd if t == 0 else nc.sync
                xq.dma_start(out=w[:, 0:4, 0:BC // 2],
                             in_=Xp[isl, :, 0:BC // 2])
                xq.dma_start(out=w[:, 0:4, BC // 2:],
                             in_=Xp[isl, :, BC // 2:])

                lh = lh_pool.tile([QP, NGT, BC], f16)
                if t == 0:
                    # two b=512 passes with immediate contracts so the OUT
                    # stream starts early; rearranges over Pool/ACT/SP
                    for half in range(2):
                        hsl = slice(half * 512, half * 512 + 512)
                        lattice(w, hsl, 512)
                        for g in range(NGT):
                            q = (nc.gpsimd, nc.scalar, nc.gpsimd,
                                 nc.sync)[g % 4]
                            q.dma_start(out=lh[:, g, hsl],
                                        in_=w[g * 8:(g + 1) * 8, :, hsl])
                        contract(t, lh, half * 512, 4, 1,
                                 lambda u: False,
                                 lambda u, h: nc.sync)
                elif t == NT - 1:
                    # drain tile: lattice first (DVE is the critical path),
                    # then contract(t-2) with DVE-heavy evacs (DVE is done
                    # after the lattice), tile-7 contracts with ACT evacs,
                    # OUT spread over all queues
                    lattice(w, slice(0, 512), 512)
                    for g in range(NGT):
                        q = (nc.gpsimd, nc.gpsimd, nc.gpsimd,
                             nc.sync)[g % 4]
                        q.dma_start(out=lh[:, g, 0:512],
                                    in_=w[g * 8:(g + 1) * 8, :, 0:512])
                    lattice(w, slice(512, BC), 512)
                    if prev_lh is not None:
                        contract(t - 1, prev_lh, 0, NB, 1,
                                 lambda u: u >= 4,
                                 lambda u, h: nc.sync)
                        prev_lh = None
                    for g in range(NGT):
                        q = (nc.gpsimd, nc.gpsimd, nc.gpsimd,
                             nc.sync)[g % 4]
                        q.dma_start(out=lh[:, g, 512:],
                                    in_=w[g * 8:(g + 1) * 8, :, 512:])
                    for half in range(2):
                        contract(t, lh, half * 512, 4, 1,
                                 lambda u: u % 2 == 1,
                                 lambda u, h: (nc.sync, nc.gpsimd,
                                               nc.sync, nc.scalar)[u])
                    continue
                else:
                    lattice(w, slice(0, BC), BC)
                    # partition-shuffle rearrange: per 8-i group one
                    # SBUF->SBUF DMA; src iterates (i_sub, s, b)
                    # partition-major == dest (q=(i_sub,s), b).
                    n_act = 5
                    for g in range(NGT):
                        if t == 1 and g < 2:
                            q = nc.sync          # SP has early slack
                        elif g >= NGT - n_act:
                            q = nc.scalar
                        else:
                            q = nc.gpsimd
                        q.dma_start(out=lh[:, g, :],
                                    in_=w[g * 8:(g + 1) * 8, :, :])
                if t >= 1:
                    load_ad(t + 1)

                if prev_lh is not None:
                    # 1-block PSUM units: 16 matmuls -> one 1024-wide
                    # evac -> one OUT DMA on SP
                    contract(t - 1, prev_lh, 0, NB, 1,
                             lambda u, _t=t - 1: u < (3 if _t % 2 else 2),
                             lambda u, h: nc.sync)
                prev_lh = None if t == 0 else lh
    if not nc.is_finalized():
        nc.finalize()
    return nc


def _get_nc():
    if "nc" not in _CACHE:
        _CACHE["nc"] = _build_bass()
    return _CACHE["nc"]


def kernel(X: np.ndarray, params: np.ndarray) -> np.ndarray:
    from concourse.bass_utils import run_bass_kernel_spmd

    X = np.asarray(X, dtype=np.float32)
    params = np.asarray(params, dtype=np.float32)
    AD = _build_ad(params)

    nc = _get_nc()
    in_maps = []
    for c in range(NCORES):
        Xc = np.ascontiguousarray(
            X[c * BC:(c + 1) * BC].transpose(1, 2, 0)
        ).astype(np.float16)
        in_maps.append({"X": Xc, "AD": AD})
    res = run_bass_kernel_spmd(nc, in_maps, list(range(NCORES)))
    out = np.concatenate(
        [np.asarray(res.results[c]["OUT"]) for c in range(NCORES)], axis=0
    )
    return out.astype(np.float32)


# revision 32
# speedup vs baseline: 1.6491x; 1.0079x over previous
"""Trainium2 Bass kernel for nn_BatchHighOrderActivation.

Math (Mobius rewrite of the Lovasz extension): the reference computes
    out[b,i,:] = sum_k coef_k * params[i, idx_k, :]
with sorted-diff coefs and nested top-set masks idx_k.  Since the Lovasz
extension is linear in the table F(m) = params[i,m,:], expand F over the
subset-indicator basis: F(m) = sum_{S subseteq m} a_S  (a = Mobius
transform of F).  The Lovasz extension of [S subseteq m] is min_{j in S} x_j,
so
    out[b,i,:] = sum_{S != 0} A[i,S,:] * min_{j in S} x_j
(params row 0 never appears in the reference output, so F(0) := 0 makes
the empty-set term vanish).  A is computed once on the host.

This kills the sort/subtract/relu pipeline entirely: per (b,i) only the 15
subset-minima are needed, 4 of which are x itself and 11 of which come from
6 double-width DVE min ops (pmin pairs -> triple mins -> min4).

Device layout per core (batch-sharded 8 ways, 1024 b-rows each):
  - i on PARTITIONS, b on free dim.  Per i-tile (128 i):
    W [128 i, 15 s, 1024 b] fp16; X DMAs straight into slots 0..3.
  - DVE: 6 min ops (all 2x fp16 packed mode) fill slots 4..14.
  - The layout crossing to matmul operands is a partition-shuffle DMA
    (not a PE transpose): per 8-i group,
        lhsT[q = i_sub*15+s, b] <- W[g*8+i_sub, s, b]
    one SBUF->SBUF DMA whose src (partition-major) and dest iteration
    orders match elementwise; 2KB/partition at full DMA rate.
  - PE: per group one matmul, K=120 (block-diag A table), M=128 b, N=64.
  - ACT/DVE: PSUM fp32 -> SBUF fp16 cast-evac; SP DMAs OUT (host upcasts).

Engine busy per core (v1 cost model, wall 90.9us): SP 79.1 (X-in 22 +
OUT 47 + fill/drain shares), ACT 78.1 (out-evacs + ~4.5 rearranges/tile +
AD), Pool 77.7 (~11.5 rearranges/tile + X tile 0), DVE 76.8 (lattice 50 +
evac share), PE 29 (matmuls only).  The rearrange is the invariant cost:
15 slots x 1024 i x 2KB-b-streams / 120-partition DMAs ~ 95us of queue
time spread over the three DMA-capable queues (SP/ACT/Pool); fill/drain
are softened by half-b passes on the first and last i-tiles.
"""

import sys

for _p in ("/opt/trn_rl_repo", "/root/.axon_site/_ro/trn_rl_repo"):
    if _p not in sys.path:
        sys.path.append(_p)

import numpy as np

B, I, A, O = 8192, 1024, 4, 8
NCORES = 8
BC = B // NCORES          # batch rows per core
NS = 15                   # slots: 4 singles, 6 pairs, 4 triples, 1 full
QP = 8 * NS               # contraction partitions: q = i_sub*15 + s
NT = I // 128             # i-tiles per core
NGT = 16                  # groups of 8 i per i-tile
NG = NT * NGT             # global groups

# slot -> subset mask (bit j set iff coordinate j in S)
SLOT_MASKS = [
    1, 2, 4, 8,            # singles {0} {1} {2} {3}  (= x planes, DMA'd)
    3, 12, 5, 10, 9, 6,    # pairs {01} {23} {02} {13} {03} {12}
    14, 13, 11, 7,         # triples ~0 ~1 ~2 ~3
    15,                    # full set (min4)
]

_CACHE = {}


def _build_ad(params: np.ndarray) -> np.ndarray:
    """Block-diagonal Mobius table AD[q = i_sub*15 + s, G*64 + i_sub*8 + o]."""
    a = params.astype(np.float32).copy()
    a[:, 0, :] = 0.0                       # F(empty) := 0
    for bit in range(4):
        hi = np.where((np.arange(16) >> bit) & 1 == 1)[0]
        a[:, hi, :] = a[:, hi, :] - a[:, hi ^ (1 << bit), :]

    AD = np.zeros((QP, NG, 8, O), np.float32)
    for s, m in enumerate(SLOT_MASKS):
        for i_sub in range(8):
            AD[i_sub * NS + s, :, i_sub, :] = a[
                np.arange(NG) * 8 + i_sub, m, :
            ]
    return AD.reshape(QP, NG * 8 * O).astype(np.float16)


def _build_bass():
    import concourse.bass as bass
    import concourse.mybir as mybir
    import concourse.tile as tile
    from concourse import bacc

    f16 = mybir.dt.float16
    f32 = mybir.dt.float32
    ALU = mybir.AluOpType
    AF = mybir.ActivationFunctionType

    nc = bacc.Bacc(None)
    # X pre-transposed on host to [i, j, b] per core
    Xp = nc.declare_dram_parameter("X", [I, 4, BC], f16, isOutput=False)
    ADp = nc.declare_dram_parameter("AD", [QP, NG * 64], f16, isOutput=False)
    OUTp = nc.declare_dram_parameter("OUT", [BC, I, O], f16, isOutput=True)

    NB = BC // 128            # b-blocks per i-tile

    with tile.TileContext(nc) as tc:
        with (
            tc.tile_pool(name="consts", bufs=1) as consts,
            tc.tile_pool(name="w", bufs=3) as w_pool,
            tc.tile_pool(name="lh", bufs=2) as lh_pool,
            tc.tile_pool(name="ot", bufs=8) as ot_pool,
            tc.tile_pool(name="ps", bufs=4, space="PSUM") as ps_pool,
        ):
            ad_sb = consts.tile([QP, NG * 64], f16)
            # AD loaded in per-i-tile slices, one tile ahead of its matmuls
            AD_CHUNK = NGT * 64

            def load_ad(t):
                if t >= NT:
                    return
                nc.scalar.dma_start(
                    out=ad_sb[:, t * AD_CHUNK:(t + 1) * AD_CHUNK],
                    in_=ADp[:, t * AD_CHUNK:(t + 1) * AD_CHUNK])

            load_ad(0)
            load_ad(1)

            def lattice(w, bsl, blen):
                """Fill slots 4..14 of w[:, :, bsl] with the min lattice."""
                # (pmin01, pmin23) <- min((x0,x2), (x1,x3))
                nc.vector.tensor_tensor(
                    w[:, 4:6, bsl], w[:, 0:3:2, bsl], w[:, 1:4:2, bsl],
                    ALU.min)
                # (pmin02, pmin13) <- min((x0,x1), (x2,x3))
                nc.vector.tensor_tensor(
                    w[:, 6:8, bsl], w[:, 0:2, bsl], w[:, 2:4, bsl], ALU.min)
                # (pmin03, pmin12) <- min((x0,x1), (x3,x2))
                nc.vector.tensor_tensor(
                    w[:, 8:10, bsl], w[:, 0:2, bsl], w[:, 3:1:-1, bsl],
                    ALU.min)
                # (t~0, t~1) <- min(pmin23, (x1, x0))
                nc.vector.tensor_tensor(
                    w[:, 10:12, bsl],
                    w[:, 5:6, bsl].broadcast_to([128, 2, blen]),
                    w[:, 1::-1, bsl], ALU.min)
                # (t~2, t~3) <- min(pmin01, (x3, x2))
                nc.vector.tensor_tensor(
                    w[:, 12:14, bsl],
                    w[:, 4:5, bsl].broadcast_to([128, 2, blen]),
                    w[:, 3:1:-1, bsl], ALU.min)
                # min4 <- min(pmin01, pmin23)
                nc.vector.tensor_tensor(
                    w[:, 14, bsl], w[:, 4, bsl], w[:, 5, bsl], ALU.min)

            def contract(t, lh, b0, nblk, blk, evac_dve, out_qs):
                """Matmul+evac+OUT for b-range [b0, b0+nblk*128*blk) of
                i-tile t, in PSUM units of `blk` 128-b blocks."""
                isl = slice(t * 128, (t + 1) * 128)
                for u in range(nblk):
                    pO = ps_pool.tile([128, blk, NGT, 64], f32)
                    for h in range(blk):
                        bsl = slice(b0 + (u * blk + h) * 128,
                                    b0 + (u * blk + h + 1) * 128)
                        for g in range(NGT):
                            G = t * NGT + g
                            nc.tensor.matmul(
                                pO[:, h, g, :],
                                lhsT=lh[:, g, bsl],
                                rhs=ad_sb[:, G * 64:(G + 1) * 64],
                                start=True, stop=True)
                    ot = ot_pool.tile([128, blk, NGT, 64], f16)
                    if evac_dve(u):
                        nc.vector.tensor_scalar(
                            ot.rearrange("p a g b -> p (a g b)"),
                            pO.rearrange("p a g b -> p (a g b)"),
                            0.0, None, ALU.add)
                    else:
                        nc.scalar.activation(
                            ot.rearrange("p a g b -> p (a g b)"),
                            pO.rearrange("p a g b -> p (a g b)"), AF.Copy)
                    for h in range(blk):
                        bsl = slice(b0 + (u * blk + h) * 128,
                                    b0 + (u * blk + h + 1) * 128)
                        out_qs(u, h).dma_start(
                            out=OUTp[bsl, isl, :],
                            in_=ot[:, h].rearrange(
                                "p g (i o) -> p (g i) o", o=O))

            # staged emission: contract(t-1) is emitted AFTER tile t's
            # lattice+rearrange, so every engine's program order runs
            # next-tile feed work ahead of previous-tile drain work.
            prev_lh = None
            for t in range(NT):
                isl = slice(t * 128, (t + 1) * 128)
                w = w_pool.tile([128, NS, BC], f16)
                # X -> slots 0..3.  Tile 0 loads on Pool (idle at start) in
                # halves so its lattice starts early; later tiles on SP.
                xq = nc.gpsimd if t == 0 else nc.sync
                xq.dma_start(out=w[:, 0:4, 0:BC // 2],
                             in_=Xp[isl, :, 0:BC // 2])
                xq.dma_start(out=w[:, 0:4, BC // 2:],
                             in_=Xp[isl, :, BC // 2:])

                lh = lh_pool.tile([QP, NGT, BC], f16)
                if t == 0:
                    # two b=512 passes with immediate contracts so the OUT
                    # stream starts early; rearranges over Pool/ACT/SP
                    for half in range(2):
                        hsl = slice(half * 512, half * 512 + 512)
                        lattice(w, hsl, 512)
                        for g in range(NGT):
                            q = (nc.gpsimd, nc.scalar, nc.gpsimd,
                                 nc.sync)[g % 4]
                            q.dma_start(out=lh[:, g, hsl],
                                        in_=w[g * 8:(g + 1) * 8, :, hsl])
                        contract(t, lh, half * 512, 4, 1,
                                 lambda u: False,
                                 lambda u, h: nc.sync)
                elif t == NT - 1:
                    # drain tile: lattice first (DVE is the critical path),
                    # then contract(t-2) with DVE-heavy evacs (DVE is done
                    # after the lattice), tile-7 contracts with ACT evacs,
                    # OUT spread over all queues
                    lattice(w, slice(0, 512), 512)
                    for g in range(NGT):
                        q = (nc.gpsimd, nc.gpsimd, nc.gpsimd,
                             nc.sync)[g % 4]
                        q.dma_start(out=lh[:, g, 0:512],
                                    in_=w[g * 8:(g + 1) * 8, :, 0:512])
                    lattice(w, slice(512, BC), 512)
                    if prev_lh is not None:
                        contract(t - 1, prev_lh, 0, NB, 1,
                                 lambda u: u >= 3,
                                 lambda u, h: nc.sync)
                        prev_lh = None
                    for g in range(NGT):
                        q = (nc.gpsimd, nc.gpsimd, nc.gpsimd,
                             nc.sync)[g % 4]
                        q.dma_start(out=lh[:, g, 512:],
                                    in_=w[g * 8:(g + 1) * 8, :, 512:])
                    for half in range(2):
                        contract(t, lh, half * 512, 4, 1,
                                 lambda u: u % 2 == 1,
                                 lambda u, h: (nc.sync, nc.gpsimd,
                                               nc.sync, nc.scalar)[u])
                    continue
                else:
                    lattice(w, slice(0, BC), BC)
                    # partition-shuffle rearrange: per 8-i group one
                    # SBUF->SBUF DMA; src iterates (i_sub, s, b)
                    # partition-major == dest (q=(i_sub,s), b).
                    n_act = 4 if t % 2 else 5
                    for g in range(NGT):
                        if t == 1 and g < 2:
                            q = nc.sync          # SP has early slack
                        elif g >= NGT - n_act:
                            q = nc.scalar
                        else:
                            q = nc.gpsimd
                        q.dma_start(out=lh[:, g, :],
                                    in_=w[g * 8:(g + 1) * 8, :, :])
                if t >= 1:
                    load_ad(t + 1)

                if prev_lh is not None:
                    # 1-block PSUM units: 16 matmuls -> one 1024-wide
                    # evac -> one OUT DMA on SP
                    contract(t - 1, prev_lh, 0, NB, 1,
                             lambda u, _t=t - 1: u < (3 if _t % 2 else 2),
                             lambda u, h: nc.sync)
                prev_lh = None if t == 0 else lh
    if not nc.is_finalized():
        nc.finalize()
    return nc


def _get_nc():
    if "nc" not in _CACHE:
        _CACHE["nc"] = _build_bass()
    return _CACHE["nc"]


def kernel(X: np.ndarray, params: np.ndarray) -> np.ndarray:
    from concourse.bass_utils import run_bass_kernel_spmd

    X = np.asarray(X, dtype=np.float32)
    params = np.asarray(params, dtype=np.float32)
    AD = _build_ad(params)

    nc = _get_nc()
    in_maps = []
    for c in range(NCORES):
        Xc = np.ascontiguousarray(
            X[c * BC:(c + 1) * BC].transpose(1, 2, 0)
        ).astype(np.float16)
        in_maps.append({"X": Xc, "AD": AD})
    res = run_bass_kernel_spmd(nc, in_maps, list(range(NCORES)))
    out = np.concatenate(
        [np.asarray(res.results[c]["OUT"]) for c in range(NCORES)], axis=0
    )
    return out.astype(np.float32)
